# revision 14
# baseline (speedup 1.0000x reference)
"""GNN message passing (GraphConv_CA) kernel for Trainium2 (8 NeuronCores).

Problem: embed [50000, 64] f32; edge_index [2, 800000] i64; trend [800000] f32.
Per hop (x3): msg = agg[row] * trend; agg = segment_sum(msg, col, N).
Output: [50000, 4, 64] = concat(embed, hop1, hop2, hop3) along axis 1.

Strategy (gather + slab/bucket-partitioned scatter-add):
  - Col-sharding: core c owns destination nodes [c*6250, (c+1)*6250). Edges
    are assigned to cores by col block, so every scatter-add is local to
    the owning core; an AllGather of the per-core [6250, 64] partials
    rebuilds the replicated [N, 64] agg for the next hop's gathers.
  - dma_scatter_add's CCE read-modify-write RACES on duplicate destination
    rows within one instruction (verified on HW). Fix: the k-th edge of a
    col goes to slab k % NSLAB and bucket k // NSLAB; a scatter instruction
    covers one slab where each col appears once per bucket, and buckets
    write to disjoint row ranges (bucket*NLOC + col) of a [V*NLOC+128, 64]
    buffer. Scatter instructions targeting the same buffer serialize via
    the Tile framework's WAW dependencies (full DMA completion).
  - Bucket partials are summed on-device (one strided DMA + V-1 DVE adds)
    only for hops feeding the AllGather; the final output keeps the bucket
    layout and the host sums it for free.
  - Per hop: slabs are processed in pairs [A_lo | B_lo | B_hi | A_hi] so
    one dma_gather covers both slabs' lo (resp. hi) rows (gather indices
    are int16, so rows >= 32768 come from a view shifted by N-32768), one
    DVE multiply applies trend, and 3 scatter-adds (A_lo, B, A_hi) store.
  - Padding positions gather row 0 with trend 0 and scatter 0.0 into
    trash rows (V*NLOC+) so every index is valid and all cores share one
    static program.
  - Instruction sizes respect SWDGE ring limits (~1000 descriptors per
    engine): gather <= 15360 indices, scatter <= 7680 (2 descs/idx).
"""

import sys

sys.path.insert(0, "/opt/trn_rl_repo")

import numpy as np

import concourse.bacc as bacc
import concourse.mybir as mybir
import concourse.tile as tile
from concourse.bass_utils import run_bass_kernel_spmd

F32 = mybir.dt.float32
I16 = mybir.dt.int16


class Cfg:
    def __init__(self, N=50000, E=800000, D=64, HOPS=3, NCORES=8,
                 ROW_SPLIT=32768, SLAB_CAP=7680, GATHER_CAP=15360):
        assert N % NCORES == 0
        self.N, self.E, self.D, self.HOPS, self.NCORES = N, E, D, HOPS, NCORES
        self.NLOC = N // NCORES
        self.RS = min(ROW_SPLIT, N)
        self.HI_OFF = N - self.RS
        self.SLAB_CAP = SLAB_CAP
        self.GATHER_CAP = GATHER_CAP
        # filled by preprocess:
        self.S = None        # [NSLAB, 2] padded sizes per (slab, kind)
        self.NSLAB = None
        self.V = None        # number of buckets
        self.DST_ROWS = None


def _wrap16(arr):
    """[L] int16 -> [128, L//16]: position i at (i%16, i//16), replicated
    to 8 groups of 16 partitions."""
    w = arr.reshape(-1, 16).T
    return np.tile(w, (8, 1)).copy()


def preprocess(embed, edge_index, trend, cfg: Cfg):
    NC, NLOC = cfg.NCORES, cfg.NLOC
    row = np.asarray(edge_index[0], dtype=np.int64)
    col = np.asarray(edge_index[1], dtype=np.int64)
    trend = np.asarray(trend, dtype=np.float32)

    # per-core edges with per-col rank
    per_core = []
    maxdeg = 1
    for c in range(NC):
        m = (col // NLOC) == c
        r, cl, t = row[m], col[m] - c * NLOC, trend[m]
        o = np.argsort(cl, kind="stable")
        r, cl, t = r[o], cl[o], t[o]
        if len(cl):
            uniq, starts, cnts = np.unique(cl, return_index=True,
                                           return_counts=True)
            rank = np.arange(len(cl)) - np.repeat(starts, cnts)
            maxdeg = max(maxdeg, int(cnts.max()))
        else:
            rank = np.zeros(0, np.int64)
        per_core.append((r, cl, t, rank, r < cfg.RS))

    # pick NSLAB so every slab fits under SLAB_CAP after per-kind padding
    nslab = max(1, -(-max(len(pc[0]) for pc in per_core) // cfg.SLAB_CAP))
    while True:
        S = np.zeros((nslab, 2), dtype=np.int64)
        for c in range(NC):
            r, cl, t, rank, lo = per_core[c]
            slab = rank % nslab
            for kind in (0, 1):
                sel = slab[lo if kind == 0 else ~lo]
                if len(sel):
                    cnts = np.bincount(sel, minlength=nslab)
                    S[:, kind] = np.maximum(S[:, kind], cnts)
        S = ((S + 127) // 128) * 128
        if int(S.sum(axis=1).max()) <= cfg.SLAB_CAP:
            break
        nslab += 1
    cfg.S = S
    cfg.NSLAB = nslab
    cfg.V = -(-maxdeg // nslab)
    cfg.DST_ROWS = cfg.V * NLOC + 128
    NPOS = int(S.sum())
    cfg.NPOS = NPOS

    # units = slab pairs; last unit may be a single slab
    units = [(2 * u, 2 * u + 1) for u in range(nslab // 2)]
    if nslab % 2:
        units.append((nslab - 1, None))
    cfg.units = units
    for a, b in units:
        glo = int(S[a, 0] + (S[b, 0] if b is not None else 0))
        ghi = int(S[a, 1] + (S[b, 1] if b is not None else 0))
        assert glo <= cfg.GATHER_CAP and ghi <= cfg.GATHER_CAP, (glo, ghi)

    embed_f = np.ascontiguousarray(embed, dtype=np.float32)
    in_maps = []
    for c in range(NC):
        r, cl, t, rank, lo = per_core[c]
        slab = rank % nslab
        bucket = rank // nslab
        eff = bucket * NLOC + cl

        scat = np.zeros(NPOS, np.int16)
        scat[:] = (cfg.V * NLOC + (np.arange(NPOS) % 128)).astype(np.int16)
        trd = np.zeros(NPOS, np.float32)
        glo_parts, ghi_parts = [], []
        pos = 0

        def emit(sl, kind):
            nonlocal pos
            Sz = int(cfg.S[sl, kind])
            if Sz == 0:
                return
            sel = (slab == sl) & (lo if kind == 0 else ~lo)
            rs = r[sel] - (0 if kind == 0 else cfg.HI_OFF)
            n = len(rs)
            idxs = np.zeros(Sz, np.int64)
            idxs[:n] = rs
            (glo_parts if kind == 0 else ghi_parts).append(
                idxs.astype(np.int16))
            scat[pos:pos + n] = eff[sel]
            trd[pos:pos + n] = t[sel]
            pos += Sz

        for a, b in units:
            emit(a, 0)
            if b is not None:
                emit(b, 0)
                emit(b, 1)
            emit(a, 1)
        assert pos == NPOS

        glo_all = (np.concatenate(glo_parts) if glo_parts
                   else np.zeros(16, np.int16))
        ghi_all = (np.concatenate(ghi_parts) if ghi_parts
                   else np.zeros(16, np.int16))
        cfg.NLO_TOT = len(glo_all)
        cfg.NHI_TOT = len(ghi_all)
        in_maps.append({
            "embed": embed_f,
            "gidx_lo": _wrap16(glo_all),
            "gidx_hi": _wrap16(ghi_all),
            "scat_idx": _wrap16(scat),
            "trend_in": trd.reshape(-1, 128).T.copy(),  # [128, NPOS//128]
        })
    return in_maps


def build(cfg: Cfg, repeat=1):
    D, NC, NLOC = cfg.D, cfg.NCORES, cfg.NLOC
    S, units, V = cfg.S, cfg.units, cfg.V
    HOPS = cfg.HOPS
    NPOS = cfg.NPOS
    NCH = NPOS // 128
    CBMAX = max(
        (int(S[a].sum() + (S[b].sum() if b is not None else 0)) // 128)
        for a, b in units)
    KCOMB = NLOC * D // 128   # per-partition combine width

    nc = bacc.Bacc("TRN2", target_bir_lowering=False, debug=False,
                   num_devices=NC)

    embed = nc.dram_tensor("embed", [cfg.N, D], F32, kind="ExternalInput")
    gidx_lo = nc.dram_tensor("gidx_lo", [128, cfg.NLO_TOT // 16], I16,
                             kind="ExternalInput")
    gidx_hi = nc.dram_tensor("gidx_hi", [128, cfg.NHI_TOT // 16], I16,
                             kind="ExternalInput")
    scat_idx = nc.dram_tensor("scat_idx", [128, NPOS // 16], I16,
                              kind="ExternalInput")
    trend_in = nc.dram_tensor("trend_in", [128, NCH], F32,
                              kind="ExternalInput")
    out3 = nc.dram_tensor("out3", [HOPS, cfg.DST_ROWS, D], F32,
                          kind="ExternalOutput")

    aggs = [embed] + [
        nc.dram_tensor(f"agg{h}", [cfg.N, D], F32, addr_space="Shared")
        for h in range(1, HOPS)
    ]
    cc_in = [nc.dram_tensor(f"ccin{h}", [NLOC, D], F32)
             for h in range(HOPS - 1)]
    rg = [list(range(NC))]

    with tile.TileContext(nc) as tc:
        with (
            tc.tile_pool(name="meta", bufs=1) as meta,
            tc.tile_pool(name="gath", bufs=2) as gpool,
            tc.tile_pool(name="scal", bufs=2) as spool,
            tc.tile_pool(name="comb", bufs=1) as cpool,
        ):
            glo_sb = meta.tile([128, cfg.NLO_TOT // 16], I16)
            nc.sync.dma_start(glo_sb[:], gidx_lo[:])
            ghi_sb = meta.tile([128, cfg.NHI_TOT // 16], I16)
            nc.sync.dma_start(ghi_sb[:], gidx_hi[:])
            scat_sb = meta.tile([128, NPOS // 16], I16)
            nc.sync.dma_start(scat_sb[:], scat_idx[:])
            trend_sb = meta.tile([128, NCH, 1], F32)
            nc.sync.dma_start(trend_sb[:],
                              trend_in[:].rearrange("p (a b) -> p a b", b=1))

            for _rep in range(repeat):
              for h in range(HOPS):
                src = aggs[h].ap()
                lo_view = src[0:cfg.RS, :]
                hi_view = src[cfg.HI_OFF:cfg.N, :]
                dst = out3.ap()[h]
                lo_off = 0
                hi_off = 0
                q0 = 0
                for a, b in units:
                    alo, ahi = int(S[a, 0]), int(S[a, 1])
                    blo = int(S[b, 0]) if b is not None else 0
                    bhi = int(S[b, 1]) if b is not None else 0
                    NLO_u, NHI_u = alo + blo, ahi + bhi
                    CB_u = (NLO_u + NHI_u) // 128
                    CLO_u = NLO_u // 128
                    gt = gpool.tile([128, CBMAX, D], F32, tag="gt")
                    if NLO_u:
                        nc.gpsimd.dma_gather(
                            gt[:, 0:CLO_u, :], lo_view,
                            glo_sb[:, lo_off // 16:(lo_off + NLO_u) // 16],
                            NLO_u, NLO_u, D, single_packet=False)
                    if NHI_u:
                        nc.gpsimd.dma_gather(
                            gt[:, CLO_u:CB_u, :], hi_view,
                            ghi_sb[:, hi_off // 16:(hi_off + NHI_u) // 16],
                            NHI_u, NHI_u, D, single_packet=False)
                    st = spool.tile([128, CBMAX, D], F32, tag="st")
                    nc.vector.tensor_tensor(
                        st[:, 0:CB_u, :], gt[:, 0:CB_u, :],
                        trend_sb[:, q0:q0 + CB_u, :].broadcast_to(
                            [128, CB_u, D]),
                        mybir.AluOpType.mult)
                    # scatters: [A_lo | B_lo B_hi | A_hi]
                    ranges = []
                    if alo:
                        ranges.append((0, alo))
                    if blo + bhi:
                        ranges.append((alo, alo + blo + bhi))
                    if ahi:
                        ranges.append((alo + blo + bhi,
                                       alo + blo + bhi + ahi))
                    for p0, p1 in ranges:
                        c0, c1 = p0 // 128, p1 // 128
                        nc.gpsimd.dma_scatter_add(
                            dst, st[:, c0:c1, :],
                            scat_sb[:, (q0 * 128 + p0) // 16:
                                    (q0 * 128 + p1) // 16],
                            p1 - p0, p1 - p0, D, single_packet=False)
                    lo_off += NLO_u
                    hi_off += NHI_u
                    q0 += CB_u
                if h < HOPS - 1:
                    # combine buckets -> cc_in, then AllGather
                    comb = cpool.tile([128, V, KCOMB], F32, tag="comb")
                    nc.sync.dma_start(
                        comb[:],
                        dst[0:V * NLOC, :].rearrange("a d -> (a d)")
                        .rearrange("(v p x) -> p v x", v=V, p=128))
                    for v in range(1, V):
                        nc.vector.tensor_tensor(
                            comb[:, 0, :], comb[:, 0, :], comb[:, v, :],
                            mybir.AluOpType.add)
                    nc.sync.dma_start(
                        cc_in[h].ap().rearrange("a b -> (a b)").rearrange(
                            "(p x) -> p x", p=128),
                        comb[:, 0, :])
                    nc.gpsimd.collective_compute(
                        "AllGather", mybir.AluOpType.bypass,
                        replica_groups=rg,
                        ins=[cc_in[h].ap().opt()],
                        outs=[aggs[h + 1].ap().opt()],
                    )
    nc.compile()
    return nc


def assemble(embed, results, cfg: Cfg):
    N, D, HOPS, NLOC, V = cfg.N, cfg.D, cfg.HOPS, cfg.NLOC, cfg.V
    out = np.empty((N, HOPS + 1, D), dtype=np.float32)
    out[:, 0, :] = np.asarray(embed, dtype=np.float32)
    for c in range(cfg.NCORES):
        o3 = np.asarray(results[c]["out3"]).reshape(HOPS, cfg.DST_ROWS, D)
        sl = slice(c * NLOC, (c + 1) * NLOC)
        for h in range(HOPS):
            acc = o3[h, 0:V * NLOC, :].reshape(V, NLOC, D).sum(axis=0)
            out[sl, h + 1, :] = acc
    return out


def run(embed, edge_index, trend, trace=False, trace_kwargs=None):
    cfg = Cfg()
    in_maps = preprocess(embed, edge_index, trend, cfg)
    nc = build(cfg)
    r = run_bass_kernel_spmd(
        nc, in_maps, core_ids=list(range(cfg.NCORES)),
        trace=trace, **(trace_kwargs or {}))
    return assemble(embed, r.results, cfg), r


def kernel(embed, edge_index, trend):
    out, _ = run(embed, edge_index, trend)
    return out
